# revision 13
# baseline (speedup 1.0000x reference)
"""Banded-causal complex attention on 8 Trainium2 NeuronCores.

Strategy: data-parallel over batch (B=8 -> 1 batch per core), full bf16
data path (fp32 PSUM accumulate; rel-err ~3e-3 vs the 2e-2 gate):
  - Q packed [Wqr|Wqi]*scale^2*temp, K packed [Wkr|-Wki]: complex score
    real part is ONE K=128 matmul per block pair. All weights ride in one
    packed tensor, split across both HW queues.
  - bf16 halves HBM traffic (7MB -> 3.4MB) and runs the PE at 1 cycle/row
    for every moving size (fp32r pays 4x below 256).
  - xT SBUF tile is piece-major [P, NCH, DCH, NSL] so every piece DMA is
    4KB-contiguous per partition on both ends (max descriptor size); each
    piece is split across the two HW queues by partition halves.
  - scores transposed: sT_kb[key c, query r] covers query blocks kb,kb+1.
  - exp on ACT (PSUM f32 in -> bf16 out), band+causal mask = one vector
    multiply with a precomputed [c,2,r] triangle mask.
  - attend is operand-swapped: o_psT[66, 256] = v_aug[kb].T @ p_kb with
    v_aug stationary (one LDWEIGHTS per key block, 256-wide moving), row
    64/65 of v_aug = 1.0 so rowsums ride along.
  - output accumulated transposed in SBUF f32 (vector copy + add per
    query block), streamed out on the sync queue in 4 chunks; host
    divides by rowsum, transposes, adds bv (softmax-avg of const = const).
"""

import numpy as np

B, S, D, KD = 8, 2048, 512, 64
P = 128              # partition size / query block
NB = S // P          # 16 query/key blocks
DCH = D // P         # 4 contraction chunks
NCH = 4              # column pieces
NSL = S // NCH       # 512 columns per piece
NCORES = 8
OC = KD + 2          # v | ones | ones
WC = P + P + KD      # packed weight columns per chunk (q|k|v)

_CACHE = {}
TRACE_KWARGS = {}    # test harness may set e.g. {"trace": True, "tmpdir": ...}


def _build_nc():
    import ml_dtypes
    import concourse.bacc as bacc
    import concourse.tile as tile
    import concourse.mybir as mybir
    from concourse.bass import ts

    f32 = mybir.dt.float32
    bf16 = mybir.dt.bfloat16
    nc = bacc.Bacc(None)

    xtr = nc.declare_dram_parameter("xtr", [P, NCH, DCH, NSL], bf16, isOutput=False)
    wall = nc.declare_dram_parameter("wall", [P, DCH, WC], bf16, isOutput=False)
    pq = nc.declare_dram_parameter("pq", [P, S], bf16, isOutput=False)
    pk = nc.declare_dram_parameter("pk", [P, S], bf16, isOutput=False)
    out = nc.declare_dram_parameter("out", [OC, S], f32, isOutput=True)

    ident = nc.inline_tensor(
        np.eye(KD, dtype=np.float32).astype(ml_dtypes.bfloat16), name="ident64"
    )
    cc, rr = np.meshgrid(np.arange(P), np.arange(P), indexing="ij")
    msk_np = np.stack([(cc <= rr), (cc >= rr)], axis=1).astype(np.float32)
    msk = nc.inline_tensor(msk_np.astype(ml_dtypes.bfloat16), name="trimask")

    with tile.TileContext(nc) as tc:
        with (
            tc.tile_pool(name="consts", bufs=1) as consts,
            tc.tile_pool(name="persist", bufs=1) as persist,
            tc.tile_pool(name="work", bufs=5) as work,
            tc.tile_pool(name="ps_proj", bufs=2, space="PSUM") as ps_proj,
            tc.tile_pool(name="ps_s", bufs=3, space="PSUM") as ps_s,
            tc.tile_pool(name="ps_o", bufs=3, space="PSUM") as ps_o,
        ):
            # warm the ACT exp table before it's on the critical path
            dummy = consts.tile([P, 2], f32)
            nc.vector.memset(dummy, 0.0)
            nc.scalar.activation(
                out=dummy, in_=dummy, func=mybir.ActivationFunctionType.Exp
            )

            w_sb = consts.tile([P, DCH, WC], bf16)
            xT_sb = persist.tile([P, NCH, DCH, NSL], bf16)
            pq_sb = persist.tile([P, S], bf16)
            pk_sb = persist.tile([P, S], bf16)

            # DMA schedule: weights and every x piece split across the two
            # HW queues by partition halves (4KB descriptors both ends),
            # pos first halves early on HW queues, second halves + consts
            # on the slow gpsimd SW queue.
            HP = P // 2
            HS = S // 2
            nc.sync.dma_start(out=w_sb[0:HP], in_=wall[0:HP])
            nc.scalar.dma_start(out=w_sb[HP:P], in_=wall[HP:P])
            # pieces 0+1 / 2+3 ride as single transfers per partition half:
            # 8KB contiguous per partition on both ends -> max DMA rate
            nc.sync.dma_start(out=xT_sb[0:HP, 0:2], in_=xtr[0:HP, 0:2])
            nc.scalar.dma_start(out=xT_sb[HP:P, 0:2], in_=xtr[HP:P, 0:2])
            nc.sync.dma_start(out=pq_sb[0:HP, 0:HS], in_=pq[0:HP, 0:HS])
            nc.scalar.dma_start(out=pq_sb[HP:P, 0:HS], in_=pq[HP:P, 0:HS])
            nc.sync.dma_start(out=pk_sb[0:HP, 0:HS], in_=pk[0:HP, 0:HS])
            nc.scalar.dma_start(out=pk_sb[HP:P, 0:HS], in_=pk[HP:P, 0:HS])
            nc.sync.dma_start(out=xT_sb[0:HP, 2:4], in_=xtr[0:HP, 2:4])
            nc.scalar.dma_start(out=xT_sb[HP:P, 2:4], in_=xtr[HP:P, 2:4])

            ident_sb = consts.tile([KD, KD], bf16)
            nc.gpsimd.dma_start(out=ident_sb, in_=ident[:])
            msk_sb = consts.tile([P, 2, P], bf16)
            nc.gpsimd.dma_start(out=msk_sb, in_=msk[:])
            nc.gpsimd.dma_start(out=pq_sb[:, HS:S], in_=pq[:, HS:S])
            nc.gpsimd.dma_start(out=pk_sb[:, HS:S], in_=pk[:, HS:S])

            # warm the PE (HAM clock gate) while the first DMA pieces land
            wdum = consts.tile([P, 2 * P], bf16)
            nc.vector.memset(wdum, 0.0)
            ps_dum = ps_s.tile([P, 2 * P], f32, tag="s", name="ps_dum")
            for _ in range(20):
                nc.tensor.matmul(
                    ps_dum, wdum[:, 0:P], wdum[:, 0 : 2 * P],
                    start=True, stop=True,
                )

            # qT padded by one zero block so every sT matmul is N=256
            qT_sb = persist.tile([P, S + P], bf16)
            kT_sb = persist.tile([P, S], bf16)
            vT_sb = persist.tile([KD, S], bf16)
            nc.vector.memset(qT_sb[:, S : S + P], 0.0)

            # v_aug[key, block, 0:64] = v; cols 64/65 = 1.0 (rowsum)
            v_aug = persist.tile([P, NB, OC], bf16)
            nc.vector.memset(v_aug[:, :, KD:OC], 1.0)

            # transposed output accumulator [66 feat, seq] f32
            oT_sb = persist.tile([OC, S], f32)

            def proj_piece(n):
                sl = slice(n * NSL, (n + 1) * NSL)
                for grp in range(3):  # 0=q, 1=k, 2=v
                    lo = (0, P, 2 * P)[grp]
                    m = P if grp < 2 else KD
                    ps = ps_proj.tile([m, NSL], f32, tag="ps", name="ps")
                    for c in range(DCH):
                        nc.tensor.matmul(
                            ps,
                            w_sb[:, c, lo : lo + m],
                            xT_sb[:, n, c, :],
                            start=(c == 0),
                            stop=(c == DCH - 1),
                        )
                    if grp == 0:
                        nc.vector.tensor_add(qT_sb[:, sl], ps, pq_sb[:, sl])
                    elif grp == 1:
                        nc.vector.tensor_add(kT_sb[:, sl], ps, pk_sb[:, sl])
                    else:
                        nc.scalar.activation(
                            out=vT_sb[:, sl], in_=ps,
                            func=mybir.ActivationFunctionType.Copy,
                        )

            def transpose_v(t):
                tp = ps_proj.tile([P, KD], bf16, tag="ps", name="tp")
                nc.tensor.transpose(tp, vT_sb[:, ts(t, P)], ident_sb)
                nc.vector.tensor_copy(v_aug[:, t, 0:KD], tp)

            p_tiles = {}
            o_tiles = {}

            def score_block(kb):
                # sT_kb[c, r]: keys of block kb vs queries of blocks kb,kb+1
                s_ps = ps_s.tile([P, 2 * P], f32, tag="s", name="s_ps")
                nc.tensor.matmul(
                    s_ps,
                    kT_sb[:, ts(kb, P)],
                    qT_sb[:, kb * P : kb * P + 2 * P],
                    start=True, stop=True,
                )
                p_sb = work.tile([P, 2, P], bf16, tag="p_sb")
                nc.scalar.activation(
                    out=p_sb, in_=s_ps.rearrange("c (h r) -> c h r", h=2),
                    func=mybir.ActivationFunctionType.Exp,
                )
                # band+causal: half 0 keeps keys c <= r (diag block qb=kb),
                # half 1 keeps c >= r (off-diag block qb=kb+1)
                nc.vector.tensor_mul(p_sb, p_sb, msk_sb)
                p_tiles[kb] = p_sb

            def attend(kb):
                # o_psT[66, 256] = v_aug[kb].T @ p_kb (stationary v_aug)
                o_ps = ps_o.tile([OC, 2 * P], f32, tag="o", name="o_ps")
                nc.tensor.matmul(
                    o_ps,
                    v_aug[:, kb, :],
                    p_tiles[kb].rearrange("c h r -> c (h r)"),
                    start=True, stop=True,
                )
                o_tiles[kb] = o_ps
                # query block kb final: deposit from kb-1 (upper half) plus
                # this block's lower half; both on vector, program order
                if kb == 0:
                    nc.vector.tensor_copy(oT_sb[:, 0:P], o_ps[:, 0:P])
                else:
                    nc.vector.tensor_add(
                        oT_sb[:, ts(kb, P)], oT_sb[:, ts(kb, P)], o_ps[:, 0:P]
                    )
                if kb < NB - 1:
                    nc.vector.tensor_copy(
                        oT_sb[:, ts(kb + 1, P)], o_ps[:, P : 2 * P]
                    )
                o_tiles.pop(kb - 1, None)
                p_tiles.pop(kb, None)
                if kb % 4 == 3:  # query blocks 4m..4m+3 final -> stream out
                    m = kb // 4
                    nc.sync.dma_start(
                        out=out[:, m * NSL : (m + 1) * NSL],
                        in_=oT_sb[:, m * NSL : (m + 1) * NSL],
                    )

            # ---- software-pipelined schedule over the 4 column pieces;
            # scores run 3 ahead of attends so the final drain is pure PE
            scored = 0
            attended = 0
            for n in range(NCH):
                proj_piece(n)
                for t in range(4 * n, 4 * (n + 1)):
                    transpose_v(t)
                target = min(4 * n + 2, NB - 1) if n < NCH - 1 else NB - 1
                while scored <= target:
                    score_block(scored)
                    scored += 1
                    if scored - attended >= 3:
                        attend(attended)
                        attended += 1
            while attended < NB:
                attend(attended)
                attended += 1

    nc.finalize()
    return nc


def _prep_core_inputs(inputs):
    import ml_dtypes

    bf = ml_dtypes.bfloat16
    g = lambda k: np.asarray(inputs[k], dtype=np.float32)
    x = g("x")
    scale = 1.0 / np.sqrt(np.float32(KD))
    temp = float(np.asarray(inputs["temperature"]).reshape(-1)[0])
    alpha = scale * temp  # folded (softmax temp) * (score scale)

    wq = np.concatenate([g("Wqr"), g("Wqi")], axis=1) * (scale * alpha)
    pqm = np.concatenate(
        [
            g("pos_qr") * alpha + g("bqr") * (scale * alpha),
            g("pos_qi") * alpha + g("bqi") * (scale * alpha),
        ],
        axis=1,
    ).T  # [128, S]
    wk = np.concatenate([g("Wkr"), -g("Wki")], axis=1)
    pkm = np.concatenate(
        [g("pos_kr") + g("bkr"), -(g("pos_ki") + g("bki"))], axis=1
    ).T

    # packed per-chunk weights [p, c, (q|k|v)]
    w_all = np.concatenate(
        [
            wq.reshape(DCH, P, P),
            wk.reshape(DCH, P, P),
            g("Wv").reshape(DCH, P, KD),
        ],
        axis=2,
    ).transpose(1, 0, 2)
    shared = {
        "wall": np.ascontiguousarray(w_all).astype(bf),
        "pq": np.ascontiguousarray(pqm).astype(bf),
        "pk": np.ascontiguousarray(pkm).astype(bf),
    }
    in_maps = []
    for b in range(NCORES):
        m = dict(shared)
        # xtr[p, n, c, j] = x[b].T[c*128+p, n*512+j] (partition-major so
        # multi-piece transfers are 8KB-contiguous per partition)
        xT_b = np.ascontiguousarray(x[b].T)
        m["xtr"] = np.ascontiguousarray(
            xT_b.reshape(DCH, P, NCH, NSL).transpose(1, 2, 0, 3)
        ).astype(bf)
        in_maps.append(m)
    return in_maps


def kernel(**inputs):
    from concourse.bass_utils import run_bass_kernel_spmd

    nc = _CACHE.get("nc")
    if nc is None:
        nc = _CACHE["nc"] = _build_nc()
    in_maps = _prep_core_inputs(inputs)
    res = run_bass_kernel_spmd(
        nc, in_maps, core_ids=list(range(NCORES)), **TRACE_KWARGS
    )
    _CACHE["last_result"] = res
    bv = np.asarray(inputs["bv"], dtype=np.float32).reshape(1, KD)
    outs = []
    for b in range(NCORES):
        o = res.results[b]["out"]  # [66, S] f32
        outs.append((o[:KD] / o[KD : KD + 1]).T + bv)
    return np.stack(outs, axis=0).astype(np.float32)
